# revision 28
# baseline (speedup 1.0000x reference)
"""ObjectDecoder kernel for Trainium2 (8 NeuronCores, data-parallel over batch).

Computes out[b, o, a, p, k] = sum_d x[b, o, d] * W[o, a, p, d, k] + bias[o, a, p, k]
  x: [16384, 16, 256] f32, W: [16, 4, 2, 256, 8] f32, b: [16, 4, 2, 8] f32
  out: [16384, 16, 4, 2, 8] f32

DMA-bound problem: per core the batch shard is 2048 rows -> 33.5 MB of x in
fp32. The 2e-2 rel-err budget is ~5000x above fp32 accuracy, so precision is
traded for HBM bytes: W and out move as bf16, and x moves as fp8 (e3m4,
pre-scaled by 2 with the inverse folded into W) or bf16 depending on X_DT.

Per-core plan (batch shard of 2048 rows):
  - W + bias load FIRST on the sync queue: they stream alone at full rate and
    land by ~7us, so the PE pipeline starts immediately (on a shared queue
    behind x they complete at ~24us and stall everything).
  - x pre-packed on host to xt[pair, p(128), (o2, k, b)] with d = k*128+p:
    one full-tile dma_start per object pair, all issued upfront on the sync
    queue; the 16 DMA engines stream them back-to-back (~26 GB/s/engine) and
    compute chases the loads. The last pair is two half-tiles (split on o2)
    so its matmuls overlap the final load instead of draining after it.
  - Per pair: 16 matmuls [K=128, M=64, N=512] (fp32 PSUM accumulate),
    k-outer order; the PE runs the two M=64 column-tiles concurrently, so
    effective matmul cost is ~231 ns.
  - Scalar engine evacuates PSUM with fused per-partition bias add to bf16;
    stores: two pairs per dma_start (8 KiB lines) except the last two pairs,
    stored separately so the final store starts early. opool bufs=4 keeps
    slow mid-stream stores (they get ~1/3 of the shared DMA bandwidth) from
    back-pressuring the activation pipeline.
"""

import os
from contextlib import ExitStack

os.environ.setdefault("JAX_PLATFORMS", "axon")

import ml_dtypes
import numpy as np

import concourse.bass as bass
import concourse.mybir as mybir
import concourse.tile as tile
from concourse import bacc
from concourse.bass_utils import run_bass_kernel_spmd

B, N_OBJ, DIM_IN, APK = 16384, 16, 256, 64
N_CORES = 8
BS = B // N_CORES          # 2048 batch rows per core
NT = 512                   # moving-operand tile (one PSUM bank of fp32)
NB = BS // NT              # 4 batch chunks per core
NP = N_OBJ // 2            # 8 object pairs
F32 = mybir.dt.float32
BF16 = mybir.dt.bfloat16
BF16_NP = ml_dtypes.bfloat16

# x on-device dtype: "bf16" (rel err ~3.4e-3) or "fp8" (e3m4, rel err ~1.4e-2,
# x scaled by 2 on host with the 1/2 folded into the bf16 W — exact).
# fp8 measured bit-identical to the ml_dtypes simulation on HW (subnormals
# honored); 1.36e-2 < 2e-2 gate on the fixed test inputs.
X_DT = os.environ.get("KDTYPE", "fp8")
X_MY = {"bf16": BF16, "fp8": mybir.dt.float8e3}[X_DT]
X_NP = {"bf16": BF16_NP, "fp8": ml_dtypes.float8_e3m4}[X_DT]
X_SCALE = {"bf16": 1.0, "fp8": 2.0}[X_DT]

_CACHE: dict = {}


def _build_nc():
    nc = bacc.Bacc("TRN2", target_bir_lowering=False, debug=False)

    xt = nc.declare_dram_parameter("xt", [NP, 128, 8192], X_MY, isOutput=False)
    # W both k-chunks in ONE tensor/load: fewer sync-DGE configs (the DGE
    # ring throttles configs past ~8 in flight).
    wt = nc.declare_dram_parameter("wt", [128, 2 * N_OBJ * APK], BF16, isOutput=False)
    bt = nc.declare_dram_parameter("bt", [128, NP], F32, isOutput=False)
    out = nc.declare_dram_parameter("out", [NP // 2, 128, 2, BS], BF16, isOutput=True)

    with tile.TileContext(nc) as tc, ExitStack() as ctx:
        wpool = ctx.enter_context(tc.tile_pool(name="w", bufs=1))
        xpool = ctx.enter_context(tc.tile_pool(name="x", bufs=NP - 1))
        hpool = ctx.enter_context(tc.tile_pool(name="xh", bufs=2))
        psum = ctx.enter_context(
            tc.tile_pool(name="ps", bufs=8, space=bass.MemorySpace.PSUM)
        )
        opool = ctx.enter_context(tc.tile_pool(name="o", bufs=4))

        # Head: W+bias in one load, then pair-0 as two o2-half tiles so the
        # o2=0 matmuls fire before the whole pair lands.
        wb = wpool.tile([128, 2 * N_OBJ * APK], BF16)
        nc.sync.dma_start(wb[:], wt[:])
        b_sb = wpool.tile([128, NP], F32)
        nc.sync.dma_start(b_sb[:], bt[:])
        p0 = []
        for h in range(2):
            q = hpool.tile([128, 4096], X_MY, name="q0", bufs=4)
            nc.sync.dma_start(q[:], xt[0, :, h * 4096 : (h + 1) * 4096])
            p0.append(q)

        # Remaining x loads upfront on the sync queue; the DMA engines stream
        # them in issue order, so pair i always lands before pair i+1. The
        # last pair is two half-tiles (one per object) so its compute
        # overlaps the tail of the stream.
        xts = [None]
        for op in range(1, NP - 1):
            t = xpool.tile([128, 8192], X_MY)
            nc.sync.dma_start(t[:], xt[op])
            xts.append(t)
        halves = []
        for h in range(2):
            th = hpool.tile([128, 4096], X_MY, name="th")
            nc.sync.dma_start(th[:], xt[NP - 1, :, h * 4096 : (h + 1) * 4096])
            halves.append(th)

        ot = None
        for op in range(NP):
            if op % 2 == 0:
                ot = opool.tile([128, 2, BS], BF16)
            pss = [psum.tile([128, NT], F32, name="ps") for n in range(NB)]

            def mov(o2, k, n):
                # moving operand: [128, NT] slice at (o2, k, n*NT)
                if op == 0:
                    off = k * BS + n * NT
                    return p0[o2][:, off : off + NT]
                if op == NP - 1:
                    off = k * BS + n * NT
                    return halves[o2][:, off : off + NT]
                off = o2 * 2 * BS + k * BS + n * NT
                return xts[op][:, off : off + NT]

            def mm(k, o2, n):
                nc.tensor.matmul(
                    pss[n][o2 * 64 : (o2 + 1) * 64, :],
                    wb[:, k * 1024 + (2 * op + o2) * APK :][:, :APK],
                    mov(o2, k, n),
                    start=(k == 0),
                    stop=(k == 1),
                )

            def act(n):
                # PSUM evacuation split across scalar and vector engines —
                # 32 evacuations on scalar alone (~27us) would out-pace the
                # PE (~29us) and serialize the drain tail.
                dst = ot[:, op % 2, n * NT : (n + 1) * NT]
                if n % 2 == 0:
                    nc.scalar.activation(
                        dst,
                        pss[n][:],
                        mybir.ActivationFunctionType.Identity,
                        bias=b_sb[:, op : op + 1],
                    )
                else:
                    nc.vector.tensor_scalar_add(dst, pss[n][:], b_sb[:, op : op + 1])

            if op in (0, NP - 1):
                # o2-outer: matches quarter/half load order so compute
                # starts on the first granule
                for o2 in range(2):
                    for k in range(2):
                        for n in range(NB):
                            mm(k, o2, n)
                            if o2 == 1 and k == 1:
                                act(n)
                                if op == NP - 1 and n % 2 == 1:
                                    # store per batch-half (2 KiB lines) so
                                    # the drain tail is one act + 0.26 MB
                                    hs = (n - 1) * NT
                                    nc.sync.dma_start(
                                        out[op // 2, :, 1, hs : hs + 2 * NT],
                                        ot[:, 1, hs : hs + 2 * NT],
                                    )
            else:
                for k in range(2):
                    for o2 in range(2):
                        for n in range(NB):
                            mm(k, o2, n)
                            if k == 1 and o2 == 1:
                                act(n)

            if op % 2 == 1 and op < NP - 2:
                # two pairs per store -> 8 KiB contiguous lines. These go on
                # the scalar queue: the sync queue executes dma_starts in
                # order, so a store there would head-of-line block behind
                # (or worse, delay) the whole x stream.
                nc.scalar.dma_start(out[op // 2], ot[:])
            elif op == NP - 2:
                # tail stores ride the sync queue, idle once x is done
                nc.sync.dma_start(
                    out[op // 2, :, op % 2, :], ot[:, op % 2, :]
                )

    nc.compile()
    return nc


def _get_nc():
    if "nc" not in _CACHE:
        _CACHE["nc"] = _build_nc()
    return _CACHE["nc"]


def _prep_inputs(x, W, b):
    x = np.ascontiguousarray(x, dtype=np.float32)
    # wt[p, (k, o, apk)] followed by bias[p=(o2,apk), pair], all bf16.
    # 1/X_SCALE folded in AFTER bf16 rounding (exact: exponent shift).
    wflat = (
        np.asarray(W, dtype=np.float32)
        .transpose(3, 0, 1, 2, 4)
        .reshape(2, 128, N_OBJ, APK)
        .transpose(1, 0, 2, 3)
        .astype(BF16_NP)
        .astype(np.float32)
        / X_SCALE
    ).astype(BF16_NP).reshape(128, 2 * N_OBJ * APK)
    wt = np.ascontiguousarray(wflat)
    bt = np.ascontiguousarray(
        np.asarray(b, dtype=np.float32)
        .reshape(NP, 2, APK)
        .transpose(1, 2, 0)
        .reshape(128, NP)
    )
    # x[b_all, o, d] with o = pair*2 + o2, d = k*128 + p
    # -> xt[core, pair, p, o2, k, b]
    xb = (x.reshape(N_CORES, BS, NP, 2, 2, 128) * np.float32(X_SCALE)).astype(X_NP)
    xt_all = np.ascontiguousarray(xb.transpose(0, 2, 5, 3, 4, 1)).reshape(
        N_CORES, NP, 128, 8192
    )
    return [{"xt": xt_all[c], "wt": wt, "bt": bt} for c in range(N_CORES)]


def kernel(x, W, b, _trace=False, **run_kwargs):
    nc = _get_nc()
    in_maps = _prep_inputs(x, W, b)
    res = run_bass_kernel_spmd(
        nc, in_maps, core_ids=list(range(N_CORES)), trace=_trace, **run_kwargs
    )
    _CACHE["last_results"] = res
    out = np.empty((B, N_OBJ, APK), dtype=np.float32)
    for c in range(N_CORES):
        # out_t[pp, o2*64+apk, pair2, b] -> [b, (pp,pair2,o2), apk]
        r = np.asarray(res.results[c]["out"]).astype(np.float32)
        r = r.reshape(NP // 2, 2, APK, 2, BS).transpose(4, 0, 3, 1, 2)
        out[c * BS : (c + 1) * BS] = r.reshape(BS, N_OBJ, APK)
    return out.reshape(B, N_OBJ, 4, 2, 8)


# revision 30
# speedup vs baseline: 1.1381x; 1.1381x over previous
"""ObjectDecoder kernel for Trainium2 (8 NeuronCores, data-parallel over batch).

Computes out[b, o, a, p, k] = sum_d x[b, o, d] * W[o, a, p, d, k] + bias[o, a, p, k]
  x: [16384, 16, 256] f32, W: [16, 4, 2, 256, 8] f32, b: [16, 4, 2, 8] f32
  out: [16384, 16, 4, 2, 8] f32

DMA-bound problem: per core the batch shard is 2048 rows -> 33.5 MB of x in
fp32. The 2e-2 rel-err budget is ~5000x above fp32 accuracy, so precision is
traded for HBM bytes: W and out move as bf16, and x moves as fp8 (e3m4,
pre-scaled by 2 with the inverse folded into W) or bf16 depending on X_DT.

Per-core plan (batch shard of 2048 rows):
  - W (split by k-chunk) + bias load FIRST on the sync queue, interleaved
    with pair-0 quarter-tiles, so the first matmuls fire at ~11us (behind
    the whole x stream they would wait until ~24us).
  - x pre-packed on host to xt[pair, p(128), (o2, k, b)] with d = k*128+p:
    one full-tile dma_start per object pair, all issued upfront on the sync
    queue; the 16 DMA engines stream them back-to-back (~26 GB/s/engine,
    ~420 GB/s/core) and compute chases the loads. The last pair is two
    half-tiles (split on o2) so its matmuls overlap the final load.
  - Per pair: 16 matmuls [K=128, M=64, N=512] (fp32 PSUM accumulate),
    k-outer order; the PE runs the two M=64 column-tiles concurrently, so
    effective matmul cost is ~230 ns and the PE phase is ~24us.
  - PSUM evacuation with fused per-partition bias add to bf16 alternates
    between the scalar and vector engines (either alone would pace the PE).
  - Stores ride the sync queue BEHIND the x loads: the in-order DGE then
    streams x at full rate (PE-bound, no store competition) and drains the
    stores afterwards — measured faster than overlapping stores on the
    scalar queue. Two pairs per store (8 KiB lines); the last pair stores
    per batch-half right after its evacuations so the drain tail is short.
"""

import os
from contextlib import ExitStack

os.environ.setdefault("JAX_PLATFORMS", "axon")

import ml_dtypes
import numpy as np

import concourse.bass as bass
import concourse.mybir as mybir
import concourse.tile as tile
from concourse import bacc
from concourse.bass_utils import run_bass_kernel_spmd

B, N_OBJ, DIM_IN, APK = 16384, 16, 256, 64
N_CORES = 8
BS = B // N_CORES          # 2048 batch rows per core
NT = 512                   # moving-operand tile (one PSUM bank of fp32)
NB = BS // NT              # 4 batch chunks per core
NP = N_OBJ // 2            # 8 object pairs
F32 = mybir.dt.float32
BF16 = mybir.dt.bfloat16
BF16_NP = ml_dtypes.bfloat16

# x on-device dtype: "bf16" (rel err ~3.4e-3) or "fp8" (e3m4, rel err ~1.4e-2,
# x scaled by 2 on host with the 1/2 folded into the bf16 W — exact).
# fp8 measured bit-identical to the ml_dtypes simulation on HW (subnormals
# honored); 1.36e-2 < 2e-2 gate on the fixed test inputs.
X_DT = os.environ.get("KDTYPE", "fp8")
X_MY = {"bf16": BF16, "fp8": mybir.dt.float8e3}[X_DT]
X_NP = {"bf16": BF16_NP, "fp8": ml_dtypes.float8_e3m4}[X_DT]
X_SCALE = {"bf16": 1.0, "fp8": 2.0}[X_DT]

_CACHE: dict = {}


def _build_nc():
    nc = bacc.Bacc("TRN2", target_bir_lowering=False, debug=False)

    xt = nc.declare_dram_parameter("xt", [NP, 128, 8192], X_MY, isOutput=False)
    wt = nc.declare_dram_parameter("wt", [128, 2, N_OBJ, APK], BF16, isOutput=False)
    bt = nc.declare_dram_parameter("bt", [128, NP], F32, isOutput=False)
    out = nc.declare_dram_parameter("out", [NP // 2, 128, 2, BS], BF16, isOutput=True)

    with tile.TileContext(nc) as tc, ExitStack() as ctx:
        wpool = ctx.enter_context(tc.tile_pool(name="w", bufs=1))
        xpool = ctx.enter_context(tc.tile_pool(name="x", bufs=NP - 1))
        hpool = ctx.enter_context(tc.tile_pool(name="xh", bufs=2))
        psum = ctx.enter_context(
            tc.tile_pool(name="ps", bufs=8, space=bass.MemorySpace.PSUM)
        )
        opool = ctx.enter_context(tc.tile_pool(name="o", bufs=4))

        # Head: interleave W (split by k-chunk) with pair-0 quarter-tiles so
        # the first matmuls (k=0, o2=0) fire as soon as the first ~0.8 MB
        # lands, instead of waiting for all of W plus a full pair.
        wk = [wpool.tile([128, N_OBJ, APK], BF16, name=f"wk{k}") for k in range(2)]
        quarters = {}

        def quarter_load(o2, k):
            q = hpool.tile([128, 2048], X_MY, name="q0", bufs=4)
            nc.sync.dma_start(
                q[:], xt[0, :, (o2 * 2 + k) * 2048 : (o2 * 2 + k + 1) * 2048]
            )
            quarters[o2, k] = q

        nc.sync.dma_start(wk[0][:], wt[:, 0])
        quarter_load(0, 0)
        nc.sync.dma_start(wk[1][:], wt[:, 1])
        quarter_load(0, 1)
        quarter_load(1, 0)
        quarter_load(1, 1)
        b_sb = wpool.tile([128, NP], F32)
        nc.sync.dma_start(b_sb[:], bt[:])

        # Remaining x loads upfront on the sync queue; the DMA engines stream
        # them in issue order, so pair i always lands before pair i+1. The
        # last pair is two half-tiles (one per object) so its compute
        # overlaps the tail of the stream.
        xts = [None]
        for op in range(1, NP - 1):
            t = xpool.tile([128, 8192], X_MY)
            nc.sync.dma_start(t[:], xt[op])
            xts.append(t)
        halves = []
        for h in range(2):
            th = hpool.tile([128, 4096], X_MY, name="th")
            nc.sync.dma_start(th[:], xt[NP - 1, :, h * 4096 : (h + 1) * 4096])
            halves.append(th)

        ot = None
        for op in range(NP):
            if op % 2 == 0:
                ot = opool.tile([128, 2, BS], BF16)
            pss = [psum.tile([128, NT], F32, name="ps") for n in range(NB)]

            def mov(o2, k, n):
                # moving operand: [128, NT] slice at (o2, k, n*NT)
                if op == 0:
                    return quarters[o2, k][:, n * NT : (n + 1) * NT]
                if op == NP - 1:
                    off = k * BS + n * NT
                    return halves[o2][:, off : off + NT]
                off = o2 * 2 * BS + k * BS + n * NT
                return xts[op][:, off : off + NT]

            def mm(k, o2, n):
                nc.tensor.matmul(
                    pss[n][o2 * 64 : (o2 + 1) * 64, :],
                    wk[k][:, 2 * op + o2, :],
                    mov(o2, k, n),
                    start=(k == 0),
                    stop=(k == 1),
                )

            def act(n):
                # PSUM evacuation split across scalar and vector engines —
                # 32 evacuations on scalar alone (~27us) would out-pace the
                # PE (~29us) and serialize the drain tail.
                dst = ot[:, op % 2, n * NT : (n + 1) * NT]
                if n % 2 == 0:
                    nc.scalar.activation(
                        dst,
                        pss[n][:],
                        mybir.ActivationFunctionType.Identity,
                        bias=b_sb[:, op : op + 1],
                    )
                else:
                    nc.vector.tensor_scalar_add(dst, pss[n][:], b_sb[:, op : op + 1])

            if op in (0, NP - 1):
                # o2-outer: matches quarter/half load order so compute
                # starts on the first granule
                for o2 in range(2):
                    for k in range(2):
                        for n in range(NB):
                            mm(k, o2, n)
                            if o2 == 1 and k == 1:
                                act(n)
                                if op == NP - 1 and n % 2 == 1:
                                    # store per batch-half (2 KiB lines) so
                                    # the drain tail is one act + 0.26 MB
                                    hs = (n - 1) * NT
                                    nc.sync.dma_start(
                                        out[op // 2, :, 1, hs : hs + 2 * NT],
                                        ot[:, 1, hs : hs + 2 * NT],
                                    )
            else:
                for k in range(2):
                    for o2 in range(2):
                        for n in range(NB):
                            mm(k, o2, n)
                            if k == 1 and o2 == 1:
                                act(n)

            if op % 2 == 1 and op < NP - 2:
                # two pairs per store -> 8 KiB contiguous lines. These go on
                # the scalar queue: the sync queue executes dma_starts in
                # order, so a store there would head-of-line block behind
                # (or worse, delay) the whole x stream.
                nc.scalar.dma_start(out[op // 2], ot[:])
            elif op == NP - 2:
                # tail stores ride the sync queue, idle once x is done
                nc.sync.dma_start(
                    out[op // 2, :, op % 2, :], ot[:, op % 2, :]
                )

    nc.compile()
    return nc


def _get_nc():
    if "nc" not in _CACHE:
        _CACHE["nc"] = _build_nc()
    return _CACHE["nc"]


def _prep_inputs(x, W, b):
    x = np.ascontiguousarray(x, dtype=np.float32)
    # wt[p, k, o, apk]: W[o,a,par,d,kk] -> [d,o,apk] -> [k,128,o,apk] -> [128,k,o,apk]
    # 1/X_SCALE folded in AFTER bf16 rounding (exact: exponent shift).
    wt = np.ascontiguousarray(
        (
            np.asarray(W, dtype=np.float32)
            .transpose(3, 0, 1, 2, 4)
            .reshape(2, 128, N_OBJ, APK)
            .transpose(1, 0, 2, 3)
            .astype(BF16_NP)
            .astype(np.float32)
            / X_SCALE
        ).astype(BF16_NP)
    )
    # bt[o2*64+apk, pair]
    bt = np.ascontiguousarray(
        np.asarray(b, dtype=np.float32)
        .reshape(NP, 2, APK)
        .transpose(1, 2, 0)
        .reshape(128, NP)
    )
    # x[b_all, o, d] with o = pair*2 + o2, d = k*128 + p
    # -> xt[core, pair, p, o2, k, b]
    xb = (x.reshape(N_CORES, BS, NP, 2, 2, 128) * np.float32(X_SCALE)).astype(X_NP)
    xt_all = np.ascontiguousarray(xb.transpose(0, 2, 5, 3, 4, 1)).reshape(
        N_CORES, NP, 128, 8192
    )
    return [{"xt": xt_all[c], "wt": wt, "bt": bt} for c in range(N_CORES)]


def kernel(x, W, b, _trace=False, **run_kwargs):
    nc = _get_nc()
    in_maps = _prep_inputs(x, W, b)
    res = run_bass_kernel_spmd(
        nc, in_maps, core_ids=list(range(N_CORES)), trace=_trace, **run_kwargs
    )
    _CACHE["last_results"] = res
    out = np.empty((B, N_OBJ, APK), dtype=np.float32)
    for c in range(N_CORES):
        # out_t[pp, o2*64+apk, pair2, b] -> [b, (pp,pair2,o2), apk]
        r = np.asarray(res.results[c]["out"]).astype(np.float32)
        r = r.reshape(NP // 2, 2, APK, 2, BS).transpose(4, 0, 3, 1, 2)
        out[c * BS : (c + 1) * BS] = r.reshape(BS, N_OBJ, APK)
    return out.reshape(B, N_OBJ, 4, 2, 8)
